# revision 8
# baseline (speedup 1.0000x reference)
"""DeLaN forward dynamics kernel for Trainium2 (8 NeuronCores, data-parallel).

Algorithm (mathematically equivalent to the reference, but avoids the full
7-direction Jacobian):
    c = L@b1 + Ldot@a - c2half,  with
      a  = L^T qdot
      Ldot = directional derivative of L along qdot   -> ONE forward JVP
      b1 = Ldot^T qdot
      c2half[k] = qdot^T dL/dq_k a = grad_q(qdot^T L a) -> ONE reverse VJP
    u = H@qddot + c + g,  H = L L^T

Per core: primal MLP forward in fp32 (feature-major, weights stationary, so
leaky-ReLU masks match the fp32 reference), JVP + VJP tangent passes in bf16
(1 cyc/row on the PE vs 4 for fp32; bf16 tangents are indistinguishable from
fp32 end-to-end because the fp32 reference's own kink/rounding noise
dominates), and the per-sample 7x7 tensor mechanics run batch-major on the
vector engine with multi-dim access patterns that fuse all samples of a chunk
into one instruction.
"""

import numpy as np

import concourse.bacc as bacc
import concourse.bass as bass
import concourse.mybir as mybir
import concourse.tile as tile

F32 = mybir.dt.float32
BF16 = mybir.dt.bfloat16
AF = mybir.ActivationFunctionType
ALU = mybir.AluOpType
AX = mybir.AxisListType

INP, HID, NL, LOW = 7, 512, 4, 21
SLOPE, LBIAS = 0.01, 0.1
B_FULL, N_CORES = 8192, 8
DENSE = 49  # 7x7
ROWS, COLS = np.tril_indices(INP, k=-1)


def build(nc, tc, Bc=1024, CH=256):
    """Emit the per-core program. Bc = samples per core, CH = batch-column
    chunk width (multiple of 128, <= 512)."""
    assert Bc % CH == 0 and CH % 128 == 0 and CH <= 512
    NCH = Bc // CH          # chunks
    TPC = CH // 128         # 128-sample tiles per chunk
    NT = Bc // 128          # total tiles
    KC = HID // 128         # k chunks (4)
    MC = HID // 128         # m chunks (4)

    def din(name, shape, dt=F32):
        return nc.dram_tensor(name, shape, dt, kind="ExternalInput").ap()

    def dout(name, shape, dt=F32):
        return nc.dram_tensor(name, shape, dt, kind="ExternalOutput").ap()

    state = din("state_sh", [Bc, 21])
    qfm = din("qfm", [INP, Bc])                 # q feature-major fp32
    qdfm_b = din("qdfm_b", [INP, Bc], BF16)     # qdot feature-major bf16
    wpreT = din("wpreT", [INP, HID])            # pre_w.T fp32
    wpreT_b = din("wpreT_b", [INP, HID], BF16)
    wpre_b16 = din("wpre_b16", [HID, INP], BF16)  # pre_w natural (VJP)
    wfcT = din("wfcT", [NL, HID, HID])          # fc_w[l].T fp32 (fwd lhsT)
    wfcT_b = din("wfcT_b", [NL, HID, HID], BF16)
    wfc_b = din("wfc_b", [NL, HID, HID], BF16)  # fc_w[l] natural (VJP lhsT)
    wheadT = din("wheadT", [HID, 35])           # [lo_w;ld_w;g_w].T fp32
    wheadJT_b = din("wheadJT_b", [HID, 28], BF16)
    wlift_b = din("wlift_b", [DENSE, HID], BF16)
    preb = din("preb", [HID])
    fcb = din("fcb", [NL, HID])
    headb = din("headb", [35])
    ident = din("ident", [128, 128])
    ident_b = din("ident_b", [128, 128], BF16)

    u_o = dout("u_o", [Bc, INP])
    H_o = dout("H_o", [Bc, DENSE])
    c_o = dout("c_o", [Bc, INP])
    g_o = dout("g_o", [Bc, INP])

    pools = []

    def pool(name, bufs, space="SBUF"):
        cm = tc.tile_pool(name=name, bufs=bufs, space=space)
        p = cm.__enter__()
        pools.append(cm)
        return p

    wp = pool("wp", 1)            # weights, constants (resident)
    xp = pool("xp", 3)            # primal activations per layer
    tp = pool("tp", 3)            # tangent / vjp activations
    mp = pool("mp", 1)            # masks (cyclic tags, 2 chunks in flight)
    hp = pool("hp", 2)            # head stacks
    bp = pool("bp", 1)            # batch-major mechanics buffers (resident)
    sp = pool("sp", 2)            # small scratch
    psZ = pool("psZ", 2, "PSUM")
    psT = pool("psT", 2, "PSUM")
    psH = pool("psH", 2, "PSUM")
    psM = pool("psM", 2, "PSUM")

    dma = nc.sync.dma_start

    # ---------------- resident loads ----------------
    s_bm = bp.tile([128, NT, 21], F32)
    dma(s_bm[:], state.rearrange("(t p) c -> p t c", p=128))
    qfm_s = wp.tile([INP, Bc], F32)
    dma(qfm_s[:], qfm[:])
    qdfm_s = wp.tile([INP, Bc], BF16)
    dma(qdfm_s[:], qdfm_b[:])
    wpreT_s = wp.tile([INP, HID], F32)
    dma(wpreT_s[:], wpreT[:])
    wpreT_b_s = wp.tile([INP, HID], BF16)
    dma(wpreT_b_s[:], wpreT_b[:])
    wpre_b_s = wp.tile([128, KC, INP], BF16)
    dma(wpre_b_s[:], wpre_b16.rearrange("(k p) i -> p k i", p=128))
    wfcT_s, wfcT_b_s, wfc_b_s = [], [], []
    for l in range(NL):
        t1 = wp.tile([128, KC, HID], F32, tag=f"wfcT{l}")
        dma(t1[:], wfcT[l].rearrange("(k p) o -> p k o", p=128))
        wfcT_s.append(t1)
        t2 = wp.tile([128, KC, HID], BF16, tag=f"wfcTb{l}")
        dma(t2[:], wfcT_b[l].rearrange("(k p) o -> p k o", p=128))
        wfcT_b_s.append(t2)
        t3 = wp.tile([128, KC, HID], BF16, tag=f"wfcb{l}")
        dma(t3[:], wfc_b[l].rearrange("(k p) o -> p k o", p=128))
        wfc_b_s.append(t3)
    wheadT_s = wp.tile([128, KC, 35], F32)
    dma(wheadT_s[:], wheadT.rearrange("(k p) o -> p k o", p=128))
    wheadJT_s = wp.tile([128, KC, 28], BF16)
    dma(wheadJT_s[:], wheadJT_b.rearrange("(k p) o -> p k o", p=128))
    wlift_s = wp.tile([DENSE, HID], BF16)
    dma(wlift_s[:], wlift_b[:])
    preb_s = wp.tile([128, KC], F32)
    dma(preb_s[:], preb.rearrange("(k p) -> p k", p=128))
    fcb_s = wp.tile([128, NL, KC], F32)
    dma(fcb_s[:], fcb.rearrange("l (k p) -> p l k", p=128))
    headb_s = wp.tile([35, 1], F32)
    dma(headb_s[:], headb.unsqueeze(1))
    id_s = wp.tile([128, 128], F32)
    dma(id_s[:], ident[:])
    idb_s = wp.tile([128, 128], BF16)
    dma(idb_s[:], ident_b[:])

    # ---------------- resident mechanics buffers ----------------
    tbm = bp.tile([128, NT, 96], F32)       # transposed heads, batch-major
    sig = bp.tile([128, NT, INP], F32)
    spl = bp.tile([128, NT, INP], F32)      # softplus(zld) (pre +0.1)
    denseL = bp.tile([128, NT, DENSE], F32)
    denseLd = bp.tile([128, NT, DENSE], F32)
    a_t = bp.tile([128, NT, INP], F32)
    wfull = bp.tile([128, NT, DENSE], F32)
    wfull_b = bp.tile([128, NT, DENSE], BF16)
    wfT = bp.tile([DENSE, Bc], BF16)        # wfull feature-major
    dq_fm = bp.tile([INP, Bc], F32)
    dq_bm = bp.tile([128, NT, INP], F32)
    b1_t = bp.tile([128, NT, INP], F32)
    lda_t = bp.tile([128, NT, INP], F32)
    lb1_t = bp.tile([128, NT, INP], F32)
    c_t = bp.tile([128, NT, INP], F32)
    H_t = bp.tile([128, NT, DENSE], F32)
    u_t = bp.tile([128, NT, INP], F32)
    big = bp.tile([128, TPC, 343], F32)     # scratch for H contraction

    nc.gpsimd.memset(denseL[:], 0.0)
    nc.gpsimd.memset(denseLd[:], 0.0)

    masks = [[None] * (NL + 1) for _ in range(NCH)]

    def cslice(c):
        return slice(c * CH, (c + 1) * CH)

    # ================ phase F: forward primal + JVP ================
    def phase_F(c):
        cs = cslice(c)
        x = xp.tile([128, MC, CH], F32, tag="x")
        t = tp.tile([128, MC, CH], BF16, tag="t")
        m1 = mp.tile([128, MC, CH], F32, tag=f"mask{c % 2}_0")
        masks[c][0] = m1
        for m in range(MC):
            pz = psZ.tile([128, CH], F32, tag="z")
            nc.tensor.matmul(pz[:], wpreT_s[:, m * 128:(m + 1) * 128],
                             qfm_s[:, cs], start=True, stop=True)
            nc.scalar.activation(x[:, m, :], pz[:], AF.Prelu,
                                 bias=preb_s[:, m:m + 1], scale=1.0, alpha=SLOPE)
            nc.vector.tensor_scalar(m1[:, m, :], x[:, m, :], 0.0, SLOPE,
                                    ALU.is_gt, ALU.max)
            pt = psT.tile([128, CH], F32, tag="t")
            nc.tensor.matmul(pt[:], wpreT_b_s[:, m * 128:(m + 1) * 128],
                             qdfm_s[:, cs], start=True, stop=True)
            nc.vector.tensor_tensor(t[:, m, :], pt[:], m1[:, m, :], ALU.mult)
        for l in range(NL):
            xn = xp.tile([128, MC, CH], F32, tag="x")
            tn = tp.tile([128, MC, CH], BF16, tag="t")
            mn = mp.tile([128, MC, CH], F32, tag=f"mask{c % 2}_{l + 1}")
            masks[c][l + 1] = mn
            for m in range(MC):
                pz = psZ.tile([128, CH], F32, tag="z")
                for k in range(KC):
                    nc.tensor.matmul(pz[:],
                                     wfcT_s[l][:, k, m * 128:(m + 1) * 128],
                                     x[:, k, :], start=(k == 0), stop=(k == KC - 1))
                nc.scalar.activation(xn[:, m, :], pz[:], AF.Prelu,
                                     bias=fcb_s[:, l, m:m + 1], scale=1.0,
                                     alpha=SLOPE)
                nc.vector.tensor_scalar(mn[:, m, :], xn[:, m, :], 0.0, SLOPE,
                                        ALU.is_gt, ALU.max)
                pt = psT.tile([128, CH], F32, tag="t")
                for k in range(KC):
                    nc.tensor.matmul(pt[:],
                                     wfcT_b_s[l][:, k, m * 128:(m + 1) * 128],
                                     t[:, k, :], start=(k == 0), stop=(k == KC - 1))
                nc.vector.tensor_tensor(tn[:, m, :], pt[:], mn[:, m, :], ALU.mult)
            x, t = xn, tn
        # heads: primal at rows 0:35, JVP at rows 64:92 (32-aligned starts)
        stack = hp.tile([96, CH], F32, tag="stack")
        nc.gpsimd.memset(stack[32:64, :], 0.0)
        nc.gpsimd.memset(stack[64:96, :], 0.0)
        ph = psH.tile([35, CH], F32, tag="h")
        for k in range(KC):
            nc.tensor.matmul(ph[:], wheadT_s[:, k, :], x[:, k, :],
                             start=(k == 0), stop=(k == KC - 1))
        nc.scalar.activation(stack[0:35, :], ph[:], AF.Identity,
                             bias=headb_s[:], scale=1.0)
        pj = psH.tile([28, CH], F32, tag="h")
        for k in range(KC):
            nc.tensor.matmul(pj[:], wheadJT_s[:, k, :], t[:, k, :],
                             start=(k == 0), stop=(k == KC - 1))
        nc.scalar.activation(stack[64:92, :], pj[:], AF.Copy)
        for tl in range(TPC):
            tg = c * TPC + tl
            ptr = psM.tile([128, 96], F32, tag="misc")
            nc.tensor.transpose(ptr[:], stack[:, tl * 128:(tl + 1) * 128],
                                id_s[:96, :96])
            nc.scalar.activation(tbm[:, tg, :], ptr[:], AF.Copy)
        return x, t

    # ================ phase M1: dense L, a, wfull ================
    def phase_M1(c):
        ts = slice(c * TPC, (c + 1) * TPC)
        # sig = 1/(1+exp(-zld)); spl = ln(exp(zld)+1)   (single ACT table set)
        epos = sp.tile([128, TPC, INP], F32, tag="etmp")
        nc.scalar.activation(epos[:], tbm[:, ts, 21:28], AF.Exp)
        nc.scalar.activation(spl[:, ts, :], epos[:], AF.Ln, bias=1.0, scale=1.0)
        nc.scalar.activation(epos[:], tbm[:, ts, 21:28], AF.Exp, scale=-1.0)
        nc.vector.tensor_scalar(sig[:, ts, :], epos[:], 1.0, None, ALU.add)
        nc.vector.reciprocal(sig[:, ts, :], sig[:, ts, :])
        for i in range(1, INP):
            Ti = i * (i - 1) // 2
            nc.scalar.activation(denseL[:, ts, 7 * i:7 * i + i],
                                 tbm[:, ts, Ti:Ti + i], AF.Copy)
        # diag: softplus + 0.1 at positions 8i
        nc.vector.tensor_scalar(denseL[:, ts, 0:49:8], spl[:, ts, :],
                                LBIAS, None, ALU.add)
        # a[j] = sum_i L[i,j] qdot[i]
        tmp = sp.tile([128, TPC, DENSE], F32, tag="tmp49")
        nc.vector.tensor_tensor(
            tmp[:].rearrange("p t (j i) -> p t j i", j=INP),
            denseL[:, ts, :].rearrange("p t (i j) -> p t i j", i=INP)
            .transpose([0, 1, 3, 2]),
            s_bm[:, ts, 7:14].unsqueeze(2).broadcast_to([128, TPC, INP, INP]),
            ALU.mult)
        nc.vector.tensor_reduce(
            a_t[:, ts, :], tmp[:].rearrange("p t (j i) -> p t j i", j=INP),
            axis=AX.X, op=ALU.add)
        # wfull[i,j] = qdot[i] * a[j]; diag *= sig
        nc.vector.tensor_tensor(
            wfull[:, ts, :].rearrange("p t (i j) -> p t i j", i=INP),
            s_bm[:, ts, 7:14].unsqueeze(3).broadcast_to([128, TPC, INP, INP]),
            a_t[:, ts, :].unsqueeze(2).broadcast_to([128, TPC, INP, INP]),
            ALU.mult)
        nc.vector.tensor_tensor(
            wfull[:, ts, 0:49:8], wfull[:, ts, 0:49:8], sig[:, ts, :], ALU.mult)
        nc.vector.tensor_copy(wfull_b[:, ts, :], wfull[:, ts, :])
        for tl in range(TPC):
            tg = c * TPC + tl
            pw = psM.tile([DENSE, 128], BF16, tag="misc")
            nc.tensor.transpose(pw[:], wfull_b[:, tg, :], idb_s[:])
            nc.scalar.activation(wfT[:, tg * 128:(tg + 1) * 128], pw[:], AF.Copy)

    # ================ phase B: VJP ================
    def phase_B(c):
        cs = cslice(c)
        d = tp.tile([128, MC, CH], BF16, tag="d")
        for m in range(MC):
            pd = psT.tile([128, CH], F32, tag="t")
            nc.tensor.matmul(pd[:], wlift_s[:, m * 128:(m + 1) * 128],
                             wfT[:, cs], start=True, stop=True)
            nc.vector.tensor_tensor(d[:, m, :], pd[:], masks[c][NL][:, m, :],
                                    ALU.mult)
        for l in range(NL - 1, -1, -1):
            dn = tp.tile([128, MC, CH], BF16, tag="d")
            for m in range(MC):
                pd = psT.tile([128, CH], F32, tag="t")
                for k in range(KC):
                    nc.tensor.matmul(pd[:],
                                     wfc_b_s[l][:, k, m * 128:(m + 1) * 128],
                                     d[:, k, :], start=(k == 0), stop=(k == KC - 1))
                nc.vector.tensor_tensor(dn[:, m, :], pd[:],
                                        masks[c][l][:, m, :], ALU.mult)
            d = dn
        pq = psM.tile([INP, CH], F32, tag="misc")
        for k in range(KC):
            nc.tensor.matmul(pq[:], wpre_b_s[:, k, :], d[:, k, :],
                             start=(k == 0), stop=(k == KC - 1))
        nc.scalar.activation(dq_fm[:, cs], pq[:], AF.Copy)
        for tl in range(TPC):
            tg = c * TPC + tl
            pb = psM.tile([128, INP], F32, tag="misc")
            nc.tensor.transpose(pb[:], dq_fm[:, tg * 128:(tg + 1) * 128],
                                id_s[:INP, :INP])
            nc.scalar.activation(dq_bm[:, tg, :], pb[:], AF.Copy)

    # ================ phase M2: mechanics + outputs ================
    def phase_M2(c):
        ts = slice(c * TPC, (c + 1) * TPC)
        shp = [128, TPC, INP, INP]
        for i in range(1, INP):
            Ti = i * (i - 1) // 2
            nc.scalar.activation(denseLd[:, ts, 7 * i:7 * i + i],
                                 tbm[:, ts, 64 + Ti:64 + Ti + i], AF.Copy)
        nc.vector.tensor_tensor(denseLd[:, ts, 0:49:8], sig[:, ts, :],
                                tbm[:, ts, 85:92], ALU.mult)
        tmp = sp.tile([128, TPC, DENSE], F32, tag="tmp49")
        # b1[j] = sum_i Ldot[i,j] qdot[i]
        nc.vector.tensor_tensor(
            tmp[:].rearrange("p t (j i) -> p t j i", j=INP),
            denseLd[:, ts, :].rearrange("p t (i j) -> p t i j", i=INP)
            .transpose([0, 1, 3, 2]),
            s_bm[:, ts, 7:14].unsqueeze(2).broadcast_to(shp), ALU.mult)
        nc.vector.tensor_reduce(b1_t[:, ts, :],
                                tmp[:].rearrange("p t (j i) -> p t j i", j=INP),
                                axis=AX.X, op=ALU.add)
        # Ldot_a[i] = sum_j Ldot[i,j] a[j]
        nc.vector.tensor_tensor(
            tmp[:].rearrange("p t (i j) -> p t i j", i=INP),
            denseLd[:, ts, :].rearrange("p t (i j) -> p t i j", i=INP),
            a_t[:, ts, :].unsqueeze(2).broadcast_to(shp), ALU.mult)
        nc.vector.tensor_reduce(lda_t[:, ts, :],
                                tmp[:].rearrange("p t (i j) -> p t i j", i=INP),
                                axis=AX.X, op=ALU.add)
        # Lb1[i] = sum_j L[i,j] b1[j]
        nc.vector.tensor_tensor(
            tmp[:].rearrange("p t (i j) -> p t i j", i=INP),
            denseL[:, ts, :].rearrange("p t (i j) -> p t i j", i=INP),
            b1_t[:, ts, :].unsqueeze(2).broadcast_to(shp), ALU.mult)
        nc.vector.tensor_reduce(lb1_t[:, ts, :],
                                tmp[:].rearrange("p t (i j) -> p t i j", i=INP),
                                axis=AX.X, op=ALU.add)
        nc.vector.tensor_tensor(c_t[:, ts, :], lb1_t[:, ts, :], lda_t[:, ts, :],
                                ALU.add)
        nc.vector.tensor_tensor(c_t[:, ts, :], c_t[:, ts, :], dq_bm[:, ts, :],
                                ALU.subtract)
        # H[i,m] = sum_j L[i,j] L[m,j]
        nc.vector.tensor_tensor(
            big[:].rearrange("p t (i m j) -> p t i m j", i=INP, m=INP),
            denseL[:, ts, :].rearrange("p t (i j) -> p t i j", i=INP)
            .unsqueeze(3).broadcast_to([128, TPC, INP, INP, INP]),
            denseL[:, ts, :].rearrange("p t (m j) -> p t m j", m=INP)
            .unsqueeze(2).broadcast_to([128, TPC, INP, INP, INP]),
            ALU.mult)
        nc.vector.tensor_reduce(
            H_t[:, ts, :].rearrange("p t (i m) -> p t i m", i=INP),
            big[:].rearrange("p t (i m j) -> p t i m j", i=INP, m=INP),
            axis=AX.X, op=ALU.add)
        # u = H qddot + c + g
        nc.vector.tensor_tensor(
            tmp[:].rearrange("p t (i m) -> p t i m", i=INP),
            H_t[:, ts, :].rearrange("p t (i m) -> p t i m", i=INP),
            s_bm[:, ts, 14:21].unsqueeze(2).broadcast_to(shp), ALU.mult)
        nc.vector.tensor_reduce(u_t[:, ts, :],
                                tmp[:].rearrange("p t (i m) -> p t i m", i=INP),
                                axis=AX.X, op=ALU.add)
        nc.vector.tensor_tensor(u_t[:, ts, :], u_t[:, ts, :], c_t[:, ts, :],
                                ALU.add)
        nc.vector.tensor_tensor(u_t[:, ts, :], u_t[:, ts, :], tbm[:, ts, 28:35],
                                ALU.add)
        dma(u_o.rearrange("(t p) i -> p t i", p=128)[:, ts, :], u_t[:, ts, :])
        dma(H_o.rearrange("(t p) d -> p t d", p=128)[:, ts, :], H_t[:, ts, :])
        dma(c_o.rearrange("(t p) i -> p t i", p=128)[:, ts, :], c_t[:, ts, :])
        dma(g_o.rearrange("(t p) i -> p t i", p=128)[:, ts, :], tbm[:, ts, 28:35])

    # software-pipelined emission
    for c in range(NCH):
        phase_F(c)
        if c > 0:
            phase_B(c - 1)
            phase_M2(c - 1)
        phase_M1(c)
    phase_B(NCH - 1)
    phase_M2(NCH - 1)

    for p in reversed(pools):
        p.__exit__(None, None, None)


_CACHE = {}


def _get_program(Bc=1024, CH=256):
    key = (Bc, CH)
    if key not in _CACHE:
        nc = bacc.Bacc("TRN2", target_bir_lowering=False, debug=False,
                       num_devices=1)
        with tile.TileContext(nc) as tc:
            build(nc, tc, Bc=Bc, CH=CH)
        nc.compile()
        _CACHE[key] = nc
    return _CACHE[key]


def host_prep(inputs, Bc=1024, n_cores=N_CORES):
    """Build per-core in_maps from the full inputs dict (numpy only)."""
    f32 = np.float32
    bf16 = mybir.dt.np(BF16)
    st = np.ascontiguousarray(np.asarray(inputs["state"], f32))
    pre_w = np.asarray(inputs["pre_w"], f32)
    fc_w = np.asarray(inputs["fc_w"], f32)
    lo_w = np.asarray(inputs["lo_w"], f32)
    ld_w = np.asarray(inputs["ld_w"], f32)
    g_w = np.asarray(inputs["g_w"], f32)

    wheadT = np.concatenate([lo_w, ld_w, g_w], axis=0).T.copy()
    wheadJT = np.concatenate([lo_w, ld_w], axis=0).T.copy()
    wlift = np.zeros((DENSE, HID), f32)
    for r in range(LOW):
        wlift[7 * ROWS[r] + COLS[r]] = lo_w[r]
    for i in range(INP):
        wlift[8 * i] = ld_w[i]
    headb = np.concatenate([np.asarray(inputs["lo_b"], f32),
                            np.asarray(inputs["ld_b"], f32),
                            np.asarray(inputs["g_b"], f32)])
    wfcT = np.ascontiguousarray(fc_w.transpose(0, 2, 1))

    shared = {
        "wpreT": np.ascontiguousarray(pre_w.T),
        "wpreT_b": np.ascontiguousarray(pre_w.T).astype(bf16),
        "wpre_b16": pre_w.astype(bf16),
        "wfcT": wfcT,
        "wfcT_b": wfcT.astype(bf16),
        "wfc_b": fc_w.astype(bf16),
        "wheadT": wheadT,
        "wheadJT_b": wheadJT.astype(bf16),
        "wlift_b": wlift.astype(bf16),
        "preb": np.asarray(inputs["pre_b"], f32),
        "fcb": np.asarray(inputs["fc_b"], f32),
        "headb": headb,
        "ident": np.eye(128, dtype=f32),
        "ident_b": np.eye(128, dtype=f32).astype(bf16),
    }
    in_maps = []
    for core in range(n_cores):
        sh = st[core * Bc:(core + 1) * Bc]
        m = dict(shared)
        m["state_sh"] = sh
        m["qfm"] = np.ascontiguousarray(sh[:, :INP].T)
        m["qdfm_b"] = np.ascontiguousarray(sh[:, INP:2 * INP].T).astype(bf16)
        in_maps.append(m)
    return in_maps


def gather(results):
    u = np.concatenate([r["u_o"] for r in results], axis=0)
    H = np.concatenate([r["H_o"] for r in results], axis=0).reshape(-1, INP, INP)
    c = np.concatenate([r["c_o"] for r in results], axis=0)
    g = np.concatenate([r["g_o"] for r in results], axis=0)
    return u, H, c, g


def kernel(**inputs):
    from concourse import bass_utils
    Bc = B_FULL // N_CORES
    nc = _get_program(Bc=Bc, CH=256)
    in_maps = host_prep(inputs, Bc=Bc)
    res = bass_utils.run_bass_kernel_spmd(nc, in_maps, list(range(N_CORES)))
    return gather(res.results)


# revision 18
# speedup vs baseline: 7358.4560x; 7358.4560x over previous
"""DeLaN forward dynamics kernel for Trainium2 (8 NeuronCores, data-parallel).

Algorithm (mathematically equivalent to the reference, but avoids the full
7-direction Jacobian):
    c = L@b1 + Ldot@a - c2half,  with
      a  = L^T qdot
      Ldot = directional derivative of L along qdot   -> ONE forward JVP
      b1 = Ldot^T qdot
      c2half[k] = qdot^T dL/dq_k a = grad_q(qdot^T L a) -> ONE reverse VJP
    u = H@qddot + c + g,  H = L L^T

Per core: primal MLP forward in fp32 (feature-major, weights stationary, so
leaky-ReLU masks match the fp32 reference), JVP + VJP tangent passes in bf16
(1 cyc/row on the PE vs 4 for fp32; bf16 tangents are indistinguishable from
fp32 end-to-end because the fp32 reference's own kink/rounding noise
dominates), and the per-sample 7x7 tensor mechanics run batch-major on the
vector engine with multi-dim access patterns that fuse all samples of a chunk
into one instruction.
"""

import numpy as np

import concourse.bacc as bacc
import concourse.bass as bass
import concourse.mybir as mybir
import concourse.tile as tile

F32 = mybir.dt.float32
BF16 = mybir.dt.bfloat16
AF = mybir.ActivationFunctionType
ALU = mybir.AluOpType
AX = mybir.AxisListType

INP, HID, NL, LOW = 7, 512, 4, 21
SLOPE, LBIAS = 0.01, 0.1
B_FULL, N_CORES = 8192, 8
DENSE = 49  # 7x7
ROWS, COLS = np.tril_indices(INP, k=-1)


def build(nc, tc, Bc=1024, CH=512, rep=1):
    """Emit the per-core program. Bc = samples per core, CH = batch-column
    chunk width (multiple of 128, <= 512). rep re-runs the whole compute
    (timing builds only)."""
    assert Bc % CH == 0 and CH % 128 == 0 and CH <= 512
    assert CH == 512 or CH == 128, "paired PSUM slices must own a full bank"
    NCH = Bc // CH          # chunks
    TPC = CH // 128         # 128-sample tiles per chunk
    NT = Bc // 128          # total tiles
    KC = HID // 128         # k chunks (4)
    MC = HID // 128         # m chunks (4)

    def din(name, shape, dt=F32):
        return nc.dram_tensor(name, shape, dt, kind="ExternalInput").ap()

    def dout(name, shape, dt=F32):
        return nc.dram_tensor(name, shape, dt, kind="ExternalOutput").ap()

    state = din("state_sh", [Bc, 21])
    qfm = din("qfm", [INP, Bc])                 # q feature-major fp32
    qdfm_b = din("qdfm_b", [INP, Bc], BF16)     # qdot feature-major bf16
    wpreT = din("wpreT", [INP, HID])            # pre_w.T fp32
    wpreT_b = din("wpreT_b", [INP, HID], BF16)
    wpre_b16 = din("wpre_b16", [HID, INP], BF16)  # pre_w natural (VJP)
    wfcT = din("wfcT", [NL, HID, HID])          # fc_w[l].T fp32 (fwd lhsT)
    wfcT_b = din("wfcT_b", [NL, HID, HID], BF16)
    wfc_b = din("wfc_b", [NL, HID, HID], BF16)  # fc_w[l] natural (VJP lhsT)
    wheadT = din("wheadT", [HID, 35])           # [lo_w;ld_w;g_w].T fp32
    wheadJT_b = din("wheadJT_b", [HID, 28], BF16)
    wlift_b = din("wlift_b", [DENSE, HID], BF16)
    preb = din("preb", [HID])
    fcb = din("fcb", [NL, HID])
    headb = din("headb", [35])
    ident = din("ident", [128, 128])
    ident_b = din("ident_b", [128, 128], BF16)

    u_o = dout("u_o", [Bc, INP])
    H_o = dout("H_o", [Bc, DENSE])
    c_o = dout("c_o", [Bc, INP])
    g_o = dout("g_o", [Bc, INP])

    pools = []

    def pool(name, bufs, space="SBUF"):
        cm = tc.tile_pool(name=name, bufs=bufs, space=space)
        p = cm.__enter__()
        pools.append(cm)
        return p

    wp = pool("wp", 1)            # weights, constants (resident)
    xp = pool("xp", 3)            # primal activations per layer
    tp = pool("tp", 3)            # tangent / vjp activations
    mp = pool("mp", 1)            # masks (cyclic tags, 2 chunks in flight)
    hp = pool("hp", 2)            # head stacks
    bp = pool("bp", 1)            # batch-major mechanics buffers (resident)
    sp = pool("sp", 2)            # small scratch
    psZ = pool("psZ", 2, "PSUM")
    psT = pool("psT", 2, "PSUM")
    psM = pool("psM", 2, "PSUM")
    psH = psM

    dma = nc.sync.dma_start

    # ---------------- resident loads ----------------
    # Critical-path DMAs (forward weights, in layer order) go on the HWDGE
    # queue (nc.sync); backward-pass weights aren't needed until ~100us in,
    # so they stream on the gpsimd SWDGE queue in parallel.
    dma2 = nc.gpsimd.dma_start
    qfm_s = wp.tile([INP, Bc], F32)
    dma(qfm_s[:], qfm[:])
    qdfm_s = wp.tile([INP, Bc], BF16)
    dma(qdfm_s[:], qdfm_b[:])
    wpreT_s = wp.tile([INP, HID], F32)
    dma(wpreT_s[:], wpreT[:])
    wpreT_b_s = wp.tile([INP, HID], BF16)
    dma(wpreT_b_s[:], wpreT_b[:])
    preb_s = wp.tile([128, KC], F32)
    dma(preb_s[:], preb.rearrange("(k p) -> p k", p=128))
    fcb_s = wp.tile([128, NL, KC], F32)
    dma(fcb_s[:], fcb.rearrange("l (k p) -> p l k", p=128))
    wfcT_s, wfcT_b_s, wfc_b_s = [], [], []
    for l in range(NL):
        t1 = wp.tile([128, KC, HID], F32, tag=f"wfcT{l}")
        dma(t1[:], wfcT[l].rearrange("(k p) o -> p k o", p=128))
        wfcT_s.append(t1)
        t2 = wp.tile([128, KC, HID], BF16, tag=f"wfcTb{l}")
        dma(t2[:], wfcT_b[l].rearrange("(k p) o -> p k o", p=128))
        wfcT_b_s.append(t2)
    wheadT_s = wp.tile([128, KC, 35], F32)
    dma(wheadT_s[:], wheadT.rearrange("(k p) o -> p k o", p=128))
    wheadJT_s = wp.tile([128, KC, 28], BF16)
    dma(wheadJT_s[:], wheadJT_b.rearrange("(k p) o -> p k o", p=128))
    headb_s = wp.tile([35, 1], F32)
    dma(headb_s[:], headb.unsqueeze(1))
    id_s = wp.tile([128, 128], F32)
    dma(id_s[:], ident[:])
    s_bm = bp.tile([128, NT, 21], F32)
    dma(s_bm[:], state.rearrange("(t p) c -> p t c", p=128))
    # non-critical (VJP path) on the SWDGE queue
    idb_s = wp.tile([128, 128], BF16)
    dma2(idb_s[:], ident_b[:])
    wlift_s = wp.tile([DENSE, HID], BF16)
    dma2(wlift_s[:], wlift_b[:])
    for l in range(NL):
        t3 = wp.tile([128, KC, HID], BF16, tag=f"wfcb{l}")
        dma2(t3[:], wfc_b[l].rearrange("(k p) o -> p k o", p=128))
        wfc_b_s.append(t3)
    wpre_b_s = wp.tile([128, KC, INP], BF16)
    dma2(wpre_b_s[:], wpre_b16.rearrange("(k p) i -> p k i", p=128))

    # ---------------- resident mechanics buffers ----------------
    tbm = bp.tile([128, NT, 96], F32)       # transposed heads, batch-major
    sig = bp.tile([128, NT, INP], F32)
    spl = bp.tile([128, NT, INP], F32)      # softplus(zld) (pre +0.1)
    denseL = bp.tile([128, NT, DENSE], F32)
    denseLd = bp.tile([128, NT, DENSE], F32)
    a_t = bp.tile([128, NT, INP], F32)
    wfull = bp.tile([128, NT, DENSE], F32)
    wfull_b = bp.tile([128, NT, DENSE], BF16)
    wfT = bp.tile([DENSE, Bc], BF16)        # wfull feature-major
    dq_fm = bp.tile([INP, Bc], F32)
    dq_bm = bp.tile([128, NT, INP], F32)
    b1_t = bp.tile([128, NT, INP], F32)
    lda_t = bp.tile([128, NT, INP], F32)
    lb1_t = bp.tile([128, NT, INP], F32)
    c_t = bp.tile([128, NT, INP], F32)
    H_t = bp.tile([128, NT, DENSE], F32)
    u_t = bp.tile([128, NT, INP], F32)
    big = bp.tile([128, TPC, 343], F32)     # scratch for H contraction

    nc.gpsimd.memset(denseL[:], 0.0)
    nc.gpsimd.memset(denseLd[:], 0.0)

    masks = [[None] * (NL + 1) for _ in range(NCH)]

    def cslice(c):
        return slice(c * CH, (c + 1) * CH)

    # ================ phase F: forward primal + JVP ================
    def phase_F(c):
        cs = cslice(c)
        x = xp.tile([128, MC, CH], F32, tag="x")
        t = tp.tile([128, MC, CH], BF16, tag="t")
        m1 = mp.tile([128, MC, CH], BF16, tag=f"mask{c % 2}_0")
        masks[c][0] = m1
        for m in range(MC):
            pz = psZ.tile([128, CH], F32, tag="z")
            nc.tensor.matmul(pz[:], wpreT_s[:, m * 128:(m + 1) * 128],
                             qfm_s[:, cs], start=True, stop=True)
            nc.scalar.activation(x[:, m, :], pz[:], AF.Prelu,
                                 bias=preb_s[:, m:m + 1], scale=1.0, alpha=SLOPE)
        nc.vector.tensor_scalar(m1[:], x[:], 0.0, SLOPE, ALU.is_gt, ALU.max)
        for h in range(MC // 2):
            pt = psT.tile([128, 2, CH], F32, tag="t")
            for j in range(2):
                mm = 2 * h + j
                nc.tensor.matmul(pt[:, j, :],
                                 wpreT_b_s[:, mm * 128:(mm + 1) * 128],
                                 qdfm_s[:, cs], start=True, stop=True)
            nc.vector.tensor_tensor(t[:, 2 * h:2 * h + 2, :], pt[:],
                                    m1[:, 2 * h:2 * h + 2, :], ALU.mult)
        for l in range(NL):
            xn = xp.tile([128, MC, CH], F32, tag="x")
            tn = tp.tile([128, MC, CH], BF16, tag="t")
            mn = mp.tile([128, MC, CH], BF16, tag=f"mask{c % 2}_{l + 1}")
            masks[c][l + 1] = mn
            for m in range(MC):
                pz = psZ.tile([128, CH], F32, tag="z")
                for k in range(KC):
                    nc.tensor.matmul(pz[:],
                                     wfcT_s[l][:, k, m * 128:(m + 1) * 128],
                                     x[:, k, :], start=(k == 0), stop=(k == KC - 1))
                nc.scalar.activation(xn[:, m, :], pz[:], AF.Prelu,
                                     bias=fcb_s[:, l, m:m + 1], scale=1.0,
                                     alpha=SLOPE)
            nc.vector.tensor_scalar(mn[:], xn[:], 0.0, SLOPE, ALU.is_gt, ALU.max)
            for h in range(MC // 2):
                pt = psT.tile([128, 2, CH], F32, tag="t")
                for j in range(2):
                    mm = 2 * h + j
                    for k in range(KC):
                        nc.tensor.matmul(pt[:, j, :],
                                         wfcT_b_s[l][:, k, mm * 128:(mm + 1) * 128],
                                         t[:, k, :],
                                         start=(k == 0), stop=(k == KC - 1))
                nc.vector.tensor_tensor(tn[:, 2 * h:2 * h + 2, :], pt[:],
                                        mn[:, 2 * h:2 * h + 2, :], ALU.mult)
            x, t = xn, tn
        # heads: primal at rows 0:35, JVP at rows 64:92 (32-aligned starts)
        stack = hp.tile([96, CH], F32, tag="stack")
        nc.gpsimd.memset(stack[32:64, :], 0.0)
        nc.gpsimd.memset(stack[64:96, :], 0.0)
        ph = psH.tile([35, CH], F32, tag="misc")
        for k in range(KC):
            nc.tensor.matmul(ph[:], wheadT_s[:, k, :], x[:, k, :],
                             start=(k == 0), stop=(k == KC - 1))
        nc.scalar.activation(stack[0:35, :], ph[:], AF.Identity,
                             bias=headb_s[:], scale=1.0)
        pj = psH.tile([28, CH], F32, tag="misc")
        for k in range(KC):
            nc.tensor.matmul(pj[:], wheadJT_s[:, k, :], t[:, k, :],
                             start=(k == 0), stop=(k == KC - 1))
        nc.scalar.activation(stack[64:92, :], pj[:], AF.Copy)
        for tl in range(TPC):
            tg = c * TPC + tl
            ptr = psM.tile([128, 96], F32, tag="misc")
            nc.tensor.transpose(ptr[:], stack[:, tl * 128:(tl + 1) * 128],
                                id_s[:96, :96])
            nc.scalar.activation(tbm[:, tg, :], ptr[:], AF.Copy)
        return x, t

    # ================ phase M1: dense L, a, wfull ================
    def phase_M1(c):
        ts = slice(c * TPC, (c + 1) * TPC)
        # sig = 1/(1+exp(-zld)); spl = ln(exp(zld)+1)   (single ACT table set)
        epos = sp.tile([128, TPC, INP], F32, tag="etmp")
        nc.scalar.activation(epos[:], tbm[:, ts, 21:28], AF.Exp)
        nc.scalar.activation(spl[:, ts, :], epos[:], AF.Ln, bias=1.0, scale=1.0)
        nc.scalar.activation(epos[:], tbm[:, ts, 21:28], AF.Exp, scale=-1.0)
        nc.vector.tensor_scalar(sig[:, ts, :], epos[:], 1.0, None, ALU.add)
        nc.vector.reciprocal(sig[:, ts, :], sig[:, ts, :])
        for i in range(1, INP):
            Ti = i * (i - 1) // 2
            nc.vector.tensor_copy(denseL[:, ts, 7 * i:7 * i + i],
                                  tbm[:, ts, Ti:Ti + i])
        # diag: softplus + 0.1 at positions 8i
        nc.vector.tensor_scalar(denseL[:, ts, 0:49:8], spl[:, ts, :],
                                LBIAS, None, ALU.add)
        # a[j] = sum_i L[i,j] qdot[i]
        tmp = sp.tile([128, TPC, DENSE], F32, tag="tmp49")
        nc.vector.tensor_tensor(
            tmp[:].rearrange("p t (j i) -> p t j i", j=INP),
            denseL[:, ts, :].rearrange("p t (i j) -> p t i j", i=INP)
            .transpose([0, 1, 3, 2]),
            s_bm[:, ts, 7:14].unsqueeze(2).broadcast_to([128, TPC, INP, INP]),
            ALU.mult)
        nc.vector.tensor_reduce(
            a_t[:, ts, :], tmp[:].rearrange("p t (j i) -> p t j i", j=INP),
            axis=AX.X, op=ALU.add)
        # wfull[i,j] = qdot[i] * a[j]; diag *= sig
        nc.vector.tensor_tensor(
            wfull[:, ts, :].rearrange("p t (i j) -> p t i j", i=INP),
            s_bm[:, ts, 7:14].unsqueeze(3).broadcast_to([128, TPC, INP, INP]),
            a_t[:, ts, :].unsqueeze(2).broadcast_to([128, TPC, INP, INP]),
            ALU.mult)
        nc.vector.tensor_tensor(
            wfull[:, ts, 0:49:8], wfull[:, ts, 0:49:8], sig[:, ts, :], ALU.mult)
        nc.vector.tensor_copy(wfull_b[:, ts, :], wfull[:, ts, :])
        for tl in range(TPC):
            tg = c * TPC + tl
            pw = psM.tile([DENSE, 128], BF16, tag="misc")
            nc.tensor.transpose(pw[:], wfull_b[:, tg, :], idb_s[:])
            nc.scalar.activation(wfT[:, tg * 128:(tg + 1) * 128], pw[:], AF.Copy)

    # ================ phase B: VJP ================
    def phase_B(c):
        cs = cslice(c)
        d = tp.tile([128, MC, CH], BF16, tag="d")
        for h in range(MC // 2):
            pd = psT.tile([128, 2, CH], F32, tag="t")
            for j in range(2):
                mm = 2 * h + j
                nc.tensor.matmul(pd[:, j, :], wlift_s[:, mm * 128:(mm + 1) * 128],
                                 wfT[:, cs], start=True, stop=True)
            nc.vector.tensor_tensor(d[:, 2 * h:2 * h + 2, :], pd[:],
                                    masks[c][NL][:, 2 * h:2 * h + 2, :], ALU.mult)
        for l in range(NL - 1, -1, -1):
            dn = tp.tile([128, MC, CH], BF16, tag="d")
            for h in range(MC // 2):
                pd = psT.tile([128, 2, CH], F32, tag="t")
                for j in range(2):
                    mm = 2 * h + j
                    for k in range(KC):
                        nc.tensor.matmul(pd[:, j, :],
                                         wfc_b_s[l][:, k, mm * 128:(mm + 1) * 128],
                                         d[:, k, :],
                                         start=(k == 0), stop=(k == KC - 1))
                nc.vector.tensor_tensor(dn[:, 2 * h:2 * h + 2, :], pd[:],
                                        masks[c][l][:, 2 * h:2 * h + 2, :],
                                        ALU.mult)
            d = dn
        pq = psM.tile([INP, CH], F32, tag="misc")
        for k in range(KC):
            nc.tensor.matmul(pq[:], wpre_b_s[:, k, :], d[:, k, :],
                             start=(k == 0), stop=(k == KC - 1))
        nc.scalar.activation(dq_fm[:, cs], pq[:], AF.Copy)
        for tl in range(TPC):
            tg = c * TPC + tl
            pb = psM.tile([128, INP], F32, tag="misc")
            nc.tensor.transpose(pb[:], dq_fm[:, tg * 128:(tg + 1) * 128],
                                id_s[:INP, :INP])
            nc.scalar.activation(dq_bm[:, tg, :], pb[:], AF.Copy)

    # ================ phase M2: mechanics + outputs ================
    def phase_M2(c):
        ts = slice(c * TPC, (c + 1) * TPC)
        shp = [128, TPC, INP, INP]
        for i in range(1, INP):
            Ti = i * (i - 1) // 2
            nc.vector.tensor_copy(denseLd[:, ts, 7 * i:7 * i + i],
                                  tbm[:, ts, 64 + Ti:64 + Ti + i])
        nc.vector.tensor_tensor(denseLd[:, ts, 0:49:8], sig[:, ts, :],
                                tbm[:, ts, 85:92], ALU.mult)
        tmp = sp.tile([128, TPC, DENSE], F32, tag="tmp49")
        # b1[j] = sum_i Ldot[i,j] qdot[i]
        nc.vector.tensor_tensor(
            tmp[:].rearrange("p t (j i) -> p t j i", j=INP),
            denseLd[:, ts, :].rearrange("p t (i j) -> p t i j", i=INP)
            .transpose([0, 1, 3, 2]),
            s_bm[:, ts, 7:14].unsqueeze(2).broadcast_to(shp), ALU.mult)
        nc.vector.tensor_reduce(b1_t[:, ts, :],
                                tmp[:].rearrange("p t (j i) -> p t j i", j=INP),
                                axis=AX.X, op=ALU.add)
        # Ldot_a[i] = sum_j Ldot[i,j] a[j]
        nc.vector.tensor_tensor(
            tmp[:].rearrange("p t (i j) -> p t i j", i=INP),
            denseLd[:, ts, :].rearrange("p t (i j) -> p t i j", i=INP),
            a_t[:, ts, :].unsqueeze(2).broadcast_to(shp), ALU.mult)
        nc.vector.tensor_reduce(lda_t[:, ts, :],
                                tmp[:].rearrange("p t (i j) -> p t i j", i=INP),
                                axis=AX.X, op=ALU.add)
        # Lb1[i] = sum_j L[i,j] b1[j]
        nc.vector.tensor_tensor(
            tmp[:].rearrange("p t (i j) -> p t i j", i=INP),
            denseL[:, ts, :].rearrange("p t (i j) -> p t i j", i=INP),
            b1_t[:, ts, :].unsqueeze(2).broadcast_to(shp), ALU.mult)
        nc.vector.tensor_reduce(lb1_t[:, ts, :],
                                tmp[:].rearrange("p t (i j) -> p t i j", i=INP),
                                axis=AX.X, op=ALU.add)
        nc.vector.tensor_tensor(c_t[:, ts, :], lb1_t[:, ts, :], lda_t[:, ts, :],
                                ALU.add)
        nc.vector.tensor_tensor(c_t[:, ts, :], c_t[:, ts, :], dq_bm[:, ts, :],
                                ALU.subtract)
        # H[i,m] = sum_j L[i,j] L[m,j]
        nc.vector.tensor_tensor(
            big[:].rearrange("p t (i m j) -> p t i m j", i=INP, m=INP),
            denseL[:, ts, :].rearrange("p t (i j) -> p t i j", i=INP)
            .unsqueeze(3).broadcast_to([128, TPC, INP, INP, INP]),
            denseL[:, ts, :].rearrange("p t (m j) -> p t m j", m=INP)
            .unsqueeze(2).broadcast_to([128, TPC, INP, INP, INP]),
            ALU.mult)
        nc.vector.tensor_reduce(
            H_t[:, ts, :].rearrange("p t (i m) -> p t i m", i=INP),
            big[:].rearrange("p t (i m j) -> p t i m j", i=INP, m=INP),
            axis=AX.X, op=ALU.add)
        # u = H qddot + c + g
        nc.vector.tensor_tensor(
            tmp[:].rearrange("p t (i m) -> p t i m", i=INP),
            H_t[:, ts, :].rearrange("p t (i m) -> p t i m", i=INP),
            s_bm[:, ts, 14:21].unsqueeze(2).broadcast_to(shp), ALU.mult)
        nc.vector.tensor_reduce(u_t[:, ts, :],
                                tmp[:].rearrange("p t (i m) -> p t i m", i=INP),
                                axis=AX.X, op=ALU.add)
        nc.vector.tensor_tensor(u_t[:, ts, :], u_t[:, ts, :], c_t[:, ts, :],
                                ALU.add)
        nc.vector.tensor_tensor(u_t[:, ts, :], u_t[:, ts, :], tbm[:, ts, 28:35],
                                ALU.add)
        dma(u_o.rearrange("(t p) i -> p t i", p=128)[:, ts, :], u_t[:, ts, :])
        dma(H_o.rearrange("(t p) d -> p t d", p=128)[:, ts, :], H_t[:, ts, :])
        dma(c_o.rearrange("(t p) i -> p t i", p=128)[:, ts, :], c_t[:, ts, :])
        dma(g_o.rearrange("(t p) i -> p t i", p=128)[:, ts, :], tbm[:, ts, 28:35])

    # software-pipelined emission: keep the PE two chunks ahead of the
    # per-chunk mechanics chain (M1 -> VJP -> M2) so its transposes/VJP
    # never stall the forward matmul stream.
    for _ in range(rep):
        phase_F(0)
        if NCH > 1:
            phase_F(1)
        for c in range(NCH):
            phase_M1(c)
            phase_B(c)
            phase_M2(c)
            if c + 2 < NCH:
                phase_F(c + 2)

    for p in reversed(pools):
        p.__exit__(None, None, None)


_CACHE = {}


def _get_program(Bc=1024, CH=512, rep=1):
    key = (Bc, CH, rep)
    if key not in _CACHE:
        nc = bacc.Bacc("TRN2", target_bir_lowering=False, debug=False,
                       num_devices=1)
        with tile.TileContext(nc) as tc:
            build(nc, tc, Bc=Bc, CH=CH, rep=rep)
        nc.compile()
        _CACHE[key] = nc
    return _CACHE[key]


class Runner:
    """Caches the jitted shard_map executable for a compiled program so
    repeated kernel invocations skip re-tracing/lowering (the stock
    run_bass_kernel_spmd builds a fresh jit closure per call)."""

    def __init__(self, nc, n_cores=N_CORES):
        import jax
        import concourse.mybir as mb
        from jax.sharding import Mesh, PartitionSpec
        from jax.experimental.shard_map import shard_map
        from concourse import bass2jax
        bass2jax.install_neuronx_cc_hook()
        self.n_cores = n_cores
        part_name = nc.partition_id_tensor.name if nc.partition_id_tensor else None
        in_names, out_names, out_avals = [], [], []
        for alloc in nc.m.functions[0].allocations:
            if not isinstance(alloc, mb.MemoryLocationSet):
                continue
            name = alloc.memorylocations[0].name
            if alloc.kind == "ExternalInput":
                if name == part_name:
                    continue
                in_names.append(name)
            elif alloc.kind == "ExternalOutput":
                out_names.append(name)
                out_avals.append(jax.core.ShapedArray(
                    tuple(alloc.tensor_shape), mb.dt.np(alloc.dtype)))
        self.in_names, self.out_names, self.out_avals = in_names, out_names, out_avals
        n_params = len(in_names)
        n_outs = len(out_names)

        all_in_names = list(in_names) + list(out_names)
        if part_name is not None:
            all_in_names.append(part_name)

        def _body(*args):
            operands = list(args)
            if part_name is not None:
                operands.append(bass2jax.partition_id_tensor())
            outs = bass2jax._bass_exec_p.bind(
                *operands,
                out_avals=tuple(out_avals),
                in_names=tuple(all_in_names),
                out_names=tuple(out_names),
                lowering_input_output_aliases=(),
                sim_require_finite=True,
                sim_require_nnan=True,
                nc=nc,
            )
            return tuple(outs)

        devices = jax.devices()[:n_cores]
        mesh = Mesh(np.asarray(devices), ("core",))
        self._mesh = mesh
        self._fn = jax.jit(
            shard_map(_body, mesh=mesh,
                      in_specs=(PartitionSpec("core"),) * (n_params + n_outs),
                      out_specs=(PartitionSpec("core"),) * n_outs,
                      check_rep=False),
            donate_argnums=tuple(range(n_params, n_params + n_outs)),
            keep_unused=True)

    def preload(self, in_maps):
        """Transfer concatenated inputs to the devices once; reuse across
        calls (pass the returned list as preloaded=)."""
        import jax
        from jax.sharding import NamedSharding, PartitionSpec
        concat_in = [np.concatenate([m[n] for m in in_maps], axis=0)
                     for n in self.in_names]
        sh = NamedSharding(self._mesh, PartitionSpec("core"))
        return [jax.device_put(a, sh) for a in concat_in]

    def __call__(self, in_maps=None, preloaded=None):
        import jax
        if preloaded is None:
            concat_in = [np.concatenate([m[n] for m in in_maps], axis=0)
                         for n in self.in_names]
        else:
            concat_in = preloaded
        zeros = [np.zeros((self.n_cores * a.shape[0], *a.shape[1:]), a.dtype)
                 for a in self.out_avals]
        out_arrs = self._fn(*concat_in, *zeros)
        out_arrs = jax.block_until_ready(out_arrs)
        return [{n: np.asarray(out_arrs[i]).reshape(self.n_cores,
                                                    *self.out_avals[i].shape)[c]
                 for i, n in enumerate(self.out_names)}
                for c in range(self.n_cores)]


_RUNNERS = {}


def get_runner(Bc=1024, CH=512, rep=1):
    key = (Bc, CH, rep)
    if key not in _RUNNERS:
        _RUNNERS[key] = Runner(_get_program(Bc, CH, rep))
    return _RUNNERS[key]


def host_prep(inputs, Bc=1024, n_cores=N_CORES):
    """Build per-core in_maps from the full inputs dict (numpy only)."""
    f32 = np.float32
    bf16 = mybir.dt.np(BF16)
    st = np.ascontiguousarray(np.asarray(inputs["state"], f32))
    pre_w = np.asarray(inputs["pre_w"], f32)
    fc_w = np.asarray(inputs["fc_w"], f32)
    lo_w = np.asarray(inputs["lo_w"], f32)
    ld_w = np.asarray(inputs["ld_w"], f32)
    g_w = np.asarray(inputs["g_w"], f32)

    wheadT = np.concatenate([lo_w, ld_w, g_w], axis=0).T.copy()
    wheadJT = np.concatenate([lo_w, ld_w], axis=0).T.copy()
    wlift = np.zeros((DENSE, HID), f32)
    for r in range(LOW):
        wlift[7 * ROWS[r] + COLS[r]] = lo_w[r]
    for i in range(INP):
        wlift[8 * i] = ld_w[i]
    headb = np.concatenate([np.asarray(inputs["lo_b"], f32),
                            np.asarray(inputs["ld_b"], f32),
                            np.asarray(inputs["g_b"], f32)])
    wfcT = np.ascontiguousarray(fc_w.transpose(0, 2, 1))

    shared = {
        "wpreT": np.ascontiguousarray(pre_w.T),
        "wpreT_b": np.ascontiguousarray(pre_w.T).astype(bf16),
        "wpre_b16": pre_w.astype(bf16),
        "wfcT": wfcT,
        "wfcT_b": wfcT.astype(bf16),
        "wfc_b": fc_w.astype(bf16),
        "wheadT": wheadT,
        "wheadJT_b": wheadJT.astype(bf16),
        "wlift_b": wlift.astype(bf16),
        "preb": np.asarray(inputs["pre_b"], f32),
        "fcb": np.asarray(inputs["fc_b"], f32),
        "headb": headb,
        "ident": np.eye(128, dtype=f32),
        "ident_b": np.eye(128, dtype=f32).astype(bf16),
    }
    in_maps = []
    for core in range(n_cores):
        sh = st[core * Bc:(core + 1) * Bc]
        m = dict(shared)
        m["state_sh"] = sh
        m["qfm"] = np.ascontiguousarray(sh[:, :INP].T)
        m["qdfm_b"] = np.ascontiguousarray(sh[:, INP:2 * INP].T).astype(bf16)
        in_maps.append(m)
    return in_maps


def gather(results):
    u = np.concatenate([r["u_o"] for r in results], axis=0)
    H = np.concatenate([r["H_o"] for r in results], axis=0).reshape(-1, INP, INP)
    c = np.concatenate([r["c_o"] for r in results], axis=0)
    g = np.concatenate([r["g_o"] for r in results], axis=0)
    return u, H, c, g


def kernel(**inputs):
    from concourse import bass_utils
    Bc = B_FULL // N_CORES
    nc = _get_program(Bc=Bc, CH=512, rep=1)
    in_maps = host_prep(inputs, Bc=Bc)
    res = bass_utils.run_bass_kernel_spmd(nc, in_maps, list(range(N_CORES)))
    return gather(res.results)
